# revision 7
# baseline (speedup 1.0000x reference)
"""CAM (channel attention) module kernel for Trainium2, 8-core data-parallel.

Computes, per batch b (one batch per NeuronCore):
    q = x[b].reshape(C, N)                  # C=512, N=4096
    E = q @ q.T                             # [C, C], symmetric
    att = softmax(rowmax(E) - E, axis=-1)   # == exp(rowmin(E)-E)/rowsum
    out = gamma * (att @ q) + x[b]

v5 design. Timeline model: the 8MB x load streams at ~350GB/s (~23us)
while the PE builds qT (transposes) and accumulates E = sum_k qT_k qT_k^T
underneath it (~22us of PE work); after the last chunk lands only a ~1us
E-tail + softmax chain remains before the out phase, which is pure PE
roofline (128 bf16 matmuls, FD=512, ~27.6us) with stores streaming
behind it. Keys:
  - fp32->bf16 casts run on the ACT engine (activation Copy), so the
    cast/transpose/gather pipeline spans three engines (ACT/PE/DVE) with
    no intra-FIFO blocking; per-chunk emission is safe.
  - k-slab group widths [4,8,8,4,4,4] (chunks >=256KB: smaller chunks
    expose the ~0.65us per-dma_start HWDGE sequencer issue cost as ring
    gaps); small slabs at the end keep the post-load energy tail ~1us.
  - all qT transposes on the PE (53ns/tile warm; the xbar cannot run
    during the load phase - the fabric is saturated); attT(0) also on
    the PE since it gates the first out matmul; attT(1..3) via DMA xbar
    during the out phase where the rings have slack.
  - E symmetric: only j >= i blocks computed; j < i mirrored from E[j]
    via fp32 PE transposes interleaved between the first out MM groups.
  - out drain split: ACT does psum*(gamma/s) -> SBUF (scale is a
    per-partition AP; Copy shares the exp table set so no table reload),
    DVE adds x (exact fp32). PSUM rotation is gated only by the fast ACT
    drain; softmax laggards for blocks 2-3 interleave without stalls.
  - ~24 dummy 1-col matmuls at t0 warm the HAM clock gate while chunk 0
    is in flight.
  - att left unnormalized; gamma/s rides the drain, so gamma=0 gives
    out == x exactly (fp32 x add).
"""

import sys

import numpy as np

for _p in ("/opt/trn_rl_repo",):
    if _p not in sys.path:
        sys.path.insert(0, _p)

B, C, H, W = 8, 512, 64, 64
N = H * W  # 4096
P = 128
CT = C // P  # 4 channel tiles
KT = N // P  # 32 spatial tiles
FD = 512  # matmul free-dim / PSUM bank width (fp32)
GW = [4, 8, 8, 4, 4, 4]  # k-tiles per load group (sum 32)

_CACHE = {}


def _build_bass():
    import concourse.mybir as mybir
    import concourse.tile as tile
    from concourse import bacc
    from concourse.masks import make_identity

    fp32 = mybir.dt.float32
    bf16 = mybir.dt.bfloat16
    AX = mybir.AxisListType.X
    ALU = mybir.AluOpType
    ACT_EXP = mybir.ActivationFunctionType.Exp
    ACT_COPY = mybir.ActivationFunctionType.Copy

    nc = bacc.Bacc(None, target_bir_lowering=False, debug=False)
    x_d = nc.dram_tensor("x", [C, N], fp32, kind="ExternalInput")
    g_d = nc.dram_tensor("gamma", [1], fp32, kind="ExternalInput")
    o_d = nc.dram_tensor("out", [C, N], fp32, kind="ExternalOutput")

    groups = []
    k0 = 0
    for kw in GW:
        groups.append([(c, k0, kw) for c in range(CT)])
        k0 += kw

    with tile.TileContext(nc) as tc:
        with (
            tc.tile_pool(name="persist", bufs=1) as persist,
            tc.tile_pool(name="etm", bufs=4) as etm,
            tc.tile_pool(name="mnp", bufs=4) as mnp,
            tc.tile_pool(name="sp", bufs=4) as sp,
            tc.tile_pool(name="rgp", bufs=4) as rgp,
            tc.tile_pool(name="outp", bufs=4) as outp,
            tc.tile_pool(name="epsum", bufs=4, space="PSUM") as epsum,
            tc.tile_pool(name="opsum", bufs=3, space="PSUM") as opsum,
            tc.tile_pool(name="atps", bufs=1, space="PSUM") as atps,
        ):
            gam = persist.tile([P, 1], fp32)
            ident = persist.tile([P, P], bf16)
            make_identity(nc, ident)
            ident32 = persist.tile([P, P], fp32)
            make_identity(nc, ident32)
            jt = persist.tile([P, 2], bf16)
            nc.vector.memset(jt, 0.0)
            q = persist.tile([P, CT, N], fp32)
            q_bf = persist.tile([P, CT, N], bf16)
            # k-major qT: qT[p, k, c, v] = q[c*128+v, k*128+p]; energy rhs for
            # chunk k is the contiguous [128, 512] slab qT[:, k, :, :]
            qT = persist.tile([P, KT, CT, P], bf16)
            att = persist.tile([P, CT, C], bf16)
            attT = persist.tile([P, CT, CT, P], bf16)

            nc.gpsimd.dma_start(out=gam, in_=g_d[:].to_broadcast((P, 1)))
            idx = 0
            for grp in groups:
                for c, gk0, kw in grp:
                    sl = slice(gk0 * P, (gk0 + kw) * P)
                    ring = nc.sync if idx % 2 == 0 else nc.scalar
                    ring.dma_start(
                        out=q[:, c, sl], in_=x_d[c * P : (c + 1) * P, sl]
                    )
                    idx += 1

            # HAM warmup: tiny matmuls keep the PE busy-window active while
            # chunk 0 is in flight (warm 2.4GHz clock engages after ~3.4us
            # of sustained activity).
            warm = opsum.tile([P, 2], fp32, name="warm", tag="ops")
            for _ in range(24):
                nc.tensor.matmul(warm[0:1, 0:1], lhsT=jt[:, 0:1], rhs=jt[:, 0:1])

            def cast(ch):
                c, gk0, kw = ch
                sl = slice(gk0 * P, (gk0 + kw) * P)
                nc.scalar.activation(
                    out=q_bf[:, c, sl], in_=q[:, c, sl], func=ACT_COPY
                )

            def pe_transpose(ch):
                c, gk0, kw = ch
                tp = opsum.tile([P, kw * P], bf16, name="tp", tag="ops")
                for kk in range(kw):
                    a = (gk0 + kk) * P
                    nc.tensor.transpose(
                        tp[:, kk * P : (kk + 1) * P], q_bf[:, c, a : a + P], ident
                    )
                return tp

            def gather(ch, tp):
                c, gk0, kw = ch
                nc.vector.tensor_copy(
                    out=qT[:, gk0 : gk0 + kw, c, :],
                    in_=tp.rearrange("p (k v) -> p k v", v=P),
                )

            Es = [
                epsum.tile([P, C], fp32, name=f"E{i}", tag=f"E{i}", bufs=1)
                for i in range(CT)
            ]

            def energy(lo, hi, i_only=None):
                for k in range(lo, hi):
                    for i in range(CT) if i_only is None else [i_only]:
                        nc.tensor.matmul(
                            Es[i][:, i * P :],
                            lhsT=qT[:, k, i, :],
                            rhs=qT[:, k, i:, :],
                            start=(k == 0),
                            stop=(k == KT - 1),
                        )

            # ---- load phase ----
            for gi, grp in enumerate(groups):
                for ch in grp:
                    cast(ch)
                    tp = pe_transpose(ch)
                    gather(ch, tp)
                if gi < len(groups) - 1:
                    _, gk0, kw = grp[0]
                    energy(gk0, gk0 + kw)

            # ---- tail ----
            tk0 = KT - GW[-1]
            mns = [mnp.tile([P, 1], fp32, name=f"mn{i}", tag="mn") for i in range(CT)]
            sss = [sp.tile([P, 1], fp32, name=f"s{i}", tag="s") for i in range(CT)]
            rgs = [rgp.tile([P, 1], fp32, name=f"rg{i}", tag="rg") for i in range(CT)]

            energy(tk0, KT, i_only=0)
            energy(tk0, KT, i_only=1)
            energy(tk0, KT, i_only=2)
            nc.vector.tensor_reduce(out=mns[0], in_=Es[0], axis=AX, op=ALU.min)
            etmps = {}
            for i in range(1, CT):
                for j in range(i):
                    etmps[(i, j)] = etm.tile([P, P], fp32, name="etmp", tag="etmp")
                    nc.vector.tensor_copy(
                        out=etmps[(i, j)], in_=Es[j][:, i * P : (i + 1) * P]
                    )

            def softmax(i):
                nc.scalar.activation(
                    out=att[:, i, :],
                    in_=Es[i],
                    func=ACT_EXP,
                    bias=mns[i],
                    scale=-1.0,
                    accum_out=sss[i],
                )

            def rgcalc(i):
                nc.vector.reciprocal(out=rgs[i], in_=sss[i])
                nc.vector.tensor_mul(rgs[i], rgs[i], gam)

            softmax(0)
            # attT(0) on the PE: gates the first out matmul. Etail(3) fills
            # the PE while exp(0) runs on ACT.
            atp = atps.tile([P, CT * P], bf16, name="atp", tag="atp")
            for j in range(CT):
                nc.tensor.transpose(
                    atp[:, j * P : (j + 1) * P], att[:, 0, j * P : (j + 1) * P], ident
                )
            energy(tk0, KT, i_only=3)
            rgcalc(0)
            nc.vector.tensor_copy(
                out=attT[:, 0, :, :], in_=atp.rearrange("p (j v) -> p j v", v=P)
            )

            def mirror(i, j):
                nc.tensor.transpose(
                    Es[i][:, j * P : (j + 1) * P], etmps[(i, j)], ident32
                )

            # ---- out = gamma/s * (att @ q) + x ----
            # laggards: (engine, payload) interleaved into block 0's chunks so
            # no engine FIFO ever gates the PSUM rotation.
            def laggards(i, ch):
                if i != 0:
                    return
                if ch == 0:
                    mirror(1, 0)  # PE; after this block's first MM group
                    nc.vector.tensor_reduce(
                        out=mns[1], in_=Es[1], axis=AX, op=ALU.min
                    )
                    softmax(1)  # ACT, waits mirror(1,0) + rowmin(1)
                    nc.scalar.dma_start_transpose(
                        out=attT[:, 1, :, :], in_=att[:, 1, :]
                    )
                elif ch == 1:
                    mirror(2, 0)
                    mirror(2, 1)
                    rgcalc(1)
                    nc.vector.tensor_reduce(
                        out=mns[2], in_=Es[2], axis=AX, op=ALU.min
                    )
                elif ch == 2:
                    softmax(2)
                    nc.sync.dma_start_transpose(
                        out=attT[:, 2, :, :], in_=att[:, 2, :]
                    )
                elif ch == 3:
                    mirror(3, 0)
                    mirror(3, 1)
                    mirror(3, 2)
                    rgcalc(2)
                    nc.vector.tensor_reduce(
                        out=mns[3], in_=Es[3], axis=AX, op=ALU.min
                    )
                elif ch == 4:
                    softmax(3)
                    nc.scalar.dma_start_transpose(
                        out=attT[:, 3, :, :], in_=att[:, 3, :]
                    )
                elif ch == 5:
                    rgcalc(3)

            NCH = N // FD  # 8 chunks of 512 cols per row-block
            for i in range(CT):
                ot = None
                for ch in range(NCH):
                    sl = slice(ch * FD, (ch + 1) * FD)
                    ops = opsum.tile([P, FD], fp32, name="ops", tag="ops")
                    for j in range(CT):
                        nc.tensor.matmul(
                            ops,
                            lhsT=attT[:, i, j, :],
                            rhs=q_bf[:, j, sl],
                            start=(j == 0),
                            stop=(j == CT - 1),
                        )
                    laggards(i, ch)
                    if ch % 2 == 0:
                        ot = outp.tile([P, 2 * FD], fp32, name="ot", tag="ot")
                    hsl = slice((ch % 2) * FD, (ch % 2 + 1) * FD)
                    # drain: ot = psum * (gamma/s) on ACT, then += x on DVE
                    nc.scalar.activation(
                        out=ot[:, hsl], in_=ops, func=ACT_COPY, scale=rgs[i]
                    )
                    nc.vector.tensor_add(ot[:, hsl], ot[:, hsl], q[:, i, sl])
                    if ch % 2 == 1:
                        osl = slice((ch - 1) * FD, (ch + 1) * FD)
                        st = nc.sync if (i * NCH + ch) % 4 == 1 else nc.scalar
                        st.dma_start(out=o_d[i * P : (i + 1) * P, osl], in_=ot)

    nc.compile()
    return nc


def _get_nc():
    if "nc" not in _CACHE:
        _CACHE["nc"] = _build_bass()
    return _CACHE["nc"]


def run(x, gamma, **run_kwargs):
    """Run on 8 cores; returns (results_list, BassKernelResults)."""
    from concourse.bass_utils import run_bass_kernel_spmd

    nc = _get_nc()
    x = np.ascontiguousarray(x, dtype=np.float32)
    gamma = np.ascontiguousarray(gamma, dtype=np.float32)
    in_maps = [
        {"x": np.ascontiguousarray(x[b].reshape(C, N)), "gamma": gamma}
        for b in range(B)
    ]
    res = run_bass_kernel_spmd(nc, in_maps, core_ids=list(range(B)), **run_kwargs)
    out = np.stack([r["out"] for r in res.results]).reshape(B, C, H, W)
    return out, res


def kernel(x, gamma):
    out, _ = run(x, gamma)
    return out.astype(np.float32)


# revision 12
# speedup vs baseline: 1.1547x; 1.1547x over previous
"""CAM (channel attention) module kernel for Trainium2, 8-core data-parallel.

Computes, per batch b (one batch per NeuronCore):
    q = x[b].reshape(C, N)                  # C=512, N=4096
    E = q @ q.T                             # [C, C], symmetric
    att = softmax(rowmax(E) - E, axis=-1)   # == exp(rowmin(E)-E)/rowsum
    out = gamma * (att @ q) + x[b]

v5 design. Timeline model: the 8MB x load streams at ~350GB/s (~23us)
while the PE builds qT (transposes) and accumulates E = sum_k qT_k qT_k^T
underneath it (~22us of PE work); after the last chunk lands only a ~1us
E-tail + softmax chain remains before the out phase, which is pure PE
roofline (128 bf16 matmuls, FD=512, ~27.6us) with stores streaming
behind it. Keys:
  - fp32->bf16 casts run on the ACT engine (activation Copy), so the
    cast/transpose/gather pipeline spans three engines (ACT/PE/DVE) with
    no intra-FIFO blocking; per-chunk emission is safe.
  - k-slab group widths [4,8,8,4,4,4] (chunks >=256KB: smaller chunks
    expose the ~0.65us per-dma_start HWDGE sequencer issue cost as ring
    gaps); small slabs at the end keep the post-load energy tail ~1us.
  - all qT transposes on the PE (53ns/tile warm; the xbar cannot run
    during the load phase - the fabric is saturated); attT(0) also on
    the PE since it gates the first out matmul; attT(1..3) via DMA xbar
    during the out phase where the rings have slack.
  - E symmetric: only j >= i blocks computed; j < i mirrored from E[j]
    via fp32 PE transposes interleaved between the first out MM groups.
  - out drain split: ACT does psum*(gamma/s) -> SBUF (scale is a
    per-partition AP; Copy shares the exp table set so no table reload),
    DVE adds x (exact fp32). PSUM rotation is gated only by the fast ACT
    drain; softmax laggards for blocks 2-3 interleave without stalls.
  - ~24 dummy 1-col matmuls at t0 warm the HAM clock gate while chunk 0
    is in flight.
  - att left unnormalized; gamma/s rides the drain, so gamma=0 gives
    out == x exactly (fp32 x add).
"""

import sys

import numpy as np

for _p in ("/opt/trn_rl_repo",):
    if _p not in sys.path:
        sys.path.insert(0, _p)

B, C, H, W = 8, 512, 64, 64
N = H * W  # 4096
P = 128
CT = C // P  # 4 channel tiles
KT = N // P  # 32 spatial tiles
FD = 512  # matmul free-dim / PSUM bank width (fp32)
GW = [4, 8, 8, 4, 4, 4]  # k-tiles per load group (sum 32)

_CACHE = {}


def _build_bass():
    import concourse.mybir as mybir
    import concourse.tile as tile
    from concourse import bacc
    from concourse.masks import make_identity

    fp32 = mybir.dt.float32
    bf16 = mybir.dt.bfloat16
    AX = mybir.AxisListType.X
    ALU = mybir.AluOpType
    ACT_EXP = mybir.ActivationFunctionType.Exp
    ACT_COPY = mybir.ActivationFunctionType.Copy

    nc = bacc.Bacc(None, target_bir_lowering=False, debug=False)
    x_d = nc.dram_tensor("x", [C, N], fp32, kind="ExternalInput")
    g_d = nc.dram_tensor("gamma", [1], fp32, kind="ExternalInput")
    o_d = nc.dram_tensor("out", [C, N], fp32, kind="ExternalOutput")

    groups = []
    k0 = 0
    for kw in GW:
        groups.append([(c, k0, kw) for c in range(CT)])
        k0 += kw

    with tile.TileContext(nc) as tc:
        with (
            tc.tile_pool(name="persist", bufs=1) as persist,
            tc.tile_pool(name="etm", bufs=4) as etm,
            tc.tile_pool(name="mnp", bufs=4) as mnp,
            tc.tile_pool(name="sp", bufs=4) as sp,
            tc.tile_pool(name="rgp", bufs=4) as rgp,
            tc.tile_pool(name="outp", bufs=4) as outp,
            tc.tile_pool(name="epsum", bufs=4, space="PSUM") as epsum,
            tc.tile_pool(name="opsum", bufs=3, space="PSUM") as opsum,
            tc.tile_pool(name="atps", bufs=1, space="PSUM") as atps,
        ):
            gam = persist.tile([P, 1], fp32)
            ident = persist.tile([P, P], bf16)
            make_identity(nc, ident)
            ident32 = persist.tile([P, P], fp32)
            make_identity(nc, ident32)
            jt = persist.tile([P, 2], bf16)
            nc.vector.memset(jt, 0.0)
            q = persist.tile([P, CT, N], fp32)
            q_bf = persist.tile([P, CT, N], bf16)
            # k-major qT: qT[p, k, c, v] = q[c*128+v, k*128+p]; energy rhs for
            # chunk k is the contiguous [128, 512] slab qT[:, k, :, :]
            qT = persist.tile([P, KT, CT, P], bf16)
            att = persist.tile([P, CT, C], bf16)
            attT = persist.tile([P, CT, CT, P], bf16)

            nc.gpsimd.dma_start(out=gam, in_=g_d[:].to_broadcast((P, 1)))
            idx = 0
            for grp in groups:
                for c, gk0, kw in grp:
                    sl = slice(gk0 * P, (gk0 + kw) * P)
                    ring = nc.sync if idx % 2 == 0 else nc.scalar
                    ring.dma_start(
                        out=q[:, c, sl], in_=x_d[c * P : (c + 1) * P, sl]
                    )
                    idx += 1

            # HAM warmup: tiny matmuls keep the PE busy-window active while
            # chunk 0 is in flight (warm 2.4GHz clock engages after ~3.4us
            # of sustained activity).
            warm = opsum.tile([P, 2], fp32, name="warm", tag="ops")
            for _ in range(24):
                nc.tensor.matmul(warm[0:1, 0:1], lhsT=jt[:, 0:1], rhs=jt[:, 0:1])

            def cast(ch):
                # on DVE: the ACT engine is also the HWDGE ring sequencer and
                # blocks on dma_start ring space, so load-phase casts must not
                # share its FIFO.
                c, gk0, kw = ch
                sl = slice(gk0 * P, (gk0 + kw) * P)
                nc.vector.tensor_copy(out=q_bf[:, c, sl], in_=q[:, c, sl])

            def pe_transpose(ch):
                c, gk0, kw = ch
                tp = opsum.tile([P, kw * P], bf16, name="tp", tag="ops")
                for kk in range(kw):
                    a = (gk0 + kk) * P
                    nc.tensor.transpose(
                        tp[:, kk * P : (kk + 1) * P], q_bf[:, c, a : a + P], ident
                    )
                return tp

            def gather(ch, tp):
                c, gk0, kw = ch
                nc.vector.tensor_copy(
                    out=qT[:, gk0 : gk0 + kw, c, :],
                    in_=tp.rearrange("p (k v) -> p k v", v=P),
                )

            Es = [
                epsum.tile([P, C], fp32, name=f"E{i}", tag=f"E{i}", bufs=1)
                for i in range(CT)
            ]

            def energy(lo, hi, i_only=None):
                for k in range(lo, hi):
                    for i in range(CT) if i_only is None else [i_only]:
                        nc.tensor.matmul(
                            Es[i][:, i * P :],
                            lhsT=qT[:, k, i, :],
                            rhs=qT[:, k, i:, :],
                            start=(k == 0),
                            stop=(k == KT - 1),
                        )

            # ---- load phase (group-wise: a per-chunk interleave would chain
            # casts behind gathers in the DVE FIFO through the PE) ----
            for gi, grp in enumerate(groups):
                for ch in grp:
                    cast(ch)
                tps = [pe_transpose(ch) for ch in grp]
                for ch, tp in zip(grp, tps):
                    gather(ch, tp)
                if gi < len(groups) - 1:
                    _, gk0, kw = grp[0]
                    energy(gk0, gk0 + kw)

            # ---- tail ----
            tk0 = KT - GW[-1]
            mns = [mnp.tile([P, 1], fp32, name=f"mn{i}", tag="mn") for i in range(CT)]
            sss = [sp.tile([P, 1], fp32, name=f"s{i}", tag="s") for i in range(CT)]
            rgs = [rgp.tile([P, 1], fp32, name=f"rg{i}", tag="rg") for i in range(CT)]

            energy(tk0, KT, i_only=0)
            energy(tk0, KT, i_only=1)
            energy(tk0, KT, i_only=2)
            nc.vector.tensor_reduce(out=mns[0], in_=Es[0], axis=AX, op=ALU.min)
            etmps = {}
            for i in range(1, CT):
                for j in range(i):
                    etmps[(i, j)] = etm.tile([P, P], fp32, name="etmp", tag="etmp")
                    nc.vector.tensor_copy(
                        out=etmps[(i, j)], in_=Es[j][:, i * P : (i + 1) * P]
                    )

            def softmax(i):
                nc.scalar.activation(
                    out=att[:, i, :],
                    in_=Es[i],
                    func=ACT_EXP,
                    bias=mns[i],
                    scale=-1.0,
                    accum_out=sss[i],
                )

            def rgcalc(i):
                nc.vector.reciprocal(out=rgs[i], in_=sss[i])
                nc.vector.tensor_mul(rgs[i], rgs[i], gam)

            softmax(0)
            # attT(0) on the PE: gates the first out matmul. Etail(3) fills
            # the PE while exp(0) runs on ACT.
            atp = atps.tile([P, CT * P], bf16, name="atp", tag="atp")
            for j in range(CT):
                nc.tensor.transpose(
                    atp[:, j * P : (j + 1) * P], att[:, 0, j * P : (j + 1) * P], ident
                )
            energy(tk0, KT, i_only=3)
            rgcalc(0)
            nc.vector.tensor_copy(
                out=attT[:, 0, :, :], in_=atp.rearrange("p (j v) -> p j v", v=P)
            )

            def mirror(i, j):
                nc.tensor.transpose(
                    Es[i][:, j * P : (j + 1) * P], etmps[(i, j)], ident32
                )

            # ---- out = gamma/s * (att @ q) + x ----
            # laggards: (engine, payload) interleaved into block 0's chunks so
            # no engine FIFO ever gates the PSUM rotation.
            def laggards(i, ch):
                if i != 0:
                    return
                if ch == 0:
                    mirror(1, 0)  # PE; after this block's first MM group
                    nc.vector.tensor_reduce(
                        out=mns[1], in_=Es[1], axis=AX, op=ALU.min
                    )
                    softmax(1)  # ACT, waits mirror(1,0) + rowmin(1)
                    nc.sync.dma_start_transpose(
                        out=attT[:, 1, :, :], in_=att[:, 1, :]
                    )
                elif ch == 1:
                    mirror(2, 0)
                    mirror(2, 1)
                    rgcalc(1)
                    nc.vector.tensor_reduce(
                        out=mns[2], in_=Es[2], axis=AX, op=ALU.min
                    )
                elif ch == 2:
                    softmax(2)
                    nc.sync.dma_start_transpose(
                        out=attT[:, 2, :, :], in_=att[:, 2, :]
                    )
                elif ch == 3:
                    mirror(3, 0)
                    mirror(3, 1)
                    mirror(3, 2)
                    rgcalc(2)
                    nc.vector.tensor_reduce(
                        out=mns[3], in_=Es[3], axis=AX, op=ALU.min
                    )
                elif ch == 4:
                    softmax(3)
                    nc.sync.dma_start_transpose(
                        out=attT[:, 3, :, :], in_=att[:, 3, :]
                    )
                elif ch == 5:
                    rgcalc(3)

            NCH = N // FD  # 8 chunks of 512 cols per row-block
            for i in range(CT):
                ot = None
                for ch in range(NCH):
                    sl = slice(ch * FD, (ch + 1) * FD)
                    ops = opsum.tile([P, FD], fp32, name="ops", tag="ops")
                    for j in range(CT):
                        nc.tensor.matmul(
                            ops,
                            lhsT=attT[:, i, j, :],
                            rhs=q_bf[:, j, sl],
                            start=(j == 0),
                            stop=(j == CT - 1),
                        )
                    laggards(i, ch)
                    if ch % 2 == 0:
                        ot = outp.tile([P, 2 * FD], fp32, name="ot", tag="ot")
                    hsl = slice((ch % 2) * FD, (ch % 2 + 1) * FD)
                    # drain: ot = psum * (gamma/s) on ACT, then += x on DVE
                    nc.scalar.activation(
                        out=ot[:, hsl], in_=ops, func=ACT_COPY, scale=rgs[i]
                    )
                    nc.vector.tensor_add(ot[:, hsl], ot[:, hsl], q[:, i, sl])
                    if ch % 2 == 1:
                        # sync ring only: the scalar sequencer is the ACT
                        # engine, which must stay free for drains and exps
                        osl = slice((ch - 1) * FD, (ch + 1) * FD)
                        nc.sync.dma_start(out=o_d[i * P : (i + 1) * P, osl], in_=ot)

    nc.compile()
    return nc


def _get_nc():
    if "nc" not in _CACHE:
        _CACHE["nc"] = _build_bass()
    return _CACHE["nc"]


def run(x, gamma, **run_kwargs):
    """Run on 8 cores; returns (results_list, BassKernelResults)."""
    from concourse.bass_utils import run_bass_kernel_spmd

    nc = _get_nc()
    x = np.ascontiguousarray(x, dtype=np.float32)
    gamma = np.ascontiguousarray(gamma, dtype=np.float32)
    in_maps = [
        {"x": np.ascontiguousarray(x[b].reshape(C, N)), "gamma": gamma}
        for b in range(B)
    ]
    res = run_bass_kernel_spmd(nc, in_maps, core_ids=list(range(B)), **run_kwargs)
    out = np.stack([r["out"] for r in res.results]).reshape(B, C, H, W)
    return out, res


def kernel(x, gamma):
    out, _ = run(x, gamma)
    return out.astype(np.float32)


# revision 14
# speedup vs baseline: 1.2359x; 1.0703x over previous
"""CAM (channel attention) module kernel for Trainium2, 8-core data-parallel.

Computes, per batch b (one batch per NeuronCore):
    q = x[b].reshape(C, N)                  # C=512, N=4096
    E = q @ q.T                             # [C, C], symmetric
    att = softmax(rowmax(E) - E, axis=-1)   # == exp(rowmin(E)-E)/rowsum
    out = gamma * (att @ q) + x[b]

v5 design. Timeline model: the 8MB x load streams at ~350GB/s (~23us)
while the PE builds qT (transposes) and accumulates E = sum_k qT_k qT_k^T
underneath it (~22us of PE work); after the last chunk lands only a ~1us
E-tail + softmax chain remains before the out phase, which is pure PE
roofline (128 bf16 matmuls, FD=512, ~27.6us) with stores streaming
behind it. Keys:
  - fp32->bf16 casts run on the ACT engine (activation Copy), so the
    cast/transpose/gather pipeline spans three engines (ACT/PE/DVE) with
    no intra-FIFO blocking; per-chunk emission is safe.
  - k-slab group widths [4,8,8,4,4,4] (chunks >=256KB: smaller chunks
    expose the ~0.65us per-dma_start HWDGE sequencer issue cost as ring
    gaps); small slabs at the end keep the post-load energy tail ~1us.
  - all qT transposes on the PE (53ns/tile warm; the xbar cannot run
    during the load phase - the fabric is saturated); attT(0) also on
    the PE since it gates the first out matmul; attT(1..3) via DMA xbar
    during the out phase where the rings have slack.
  - E symmetric: only j >= i blocks computed; j < i mirrored from E[j]
    via fp32 PE transposes interleaved between the first out MM groups.
  - out drain split: ACT does psum*(gamma/s) -> SBUF (scale is a
    per-partition AP; Copy shares the exp table set so no table reload),
    DVE adds x (exact fp32). PSUM rotation is gated only by the fast ACT
    drain; softmax laggards for blocks 2-3 interleave without stalls.
  - ~24 dummy 1-col matmuls at t0 warm the HAM clock gate while chunk 0
    is in flight.
  - att left unnormalized; gamma/s rides the drain, so gamma=0 gives
    out == x exactly (fp32 x add).
"""

import sys

import numpy as np

for _p in ("/opt/trn_rl_repo",):
    if _p not in sys.path:
        sys.path.insert(0, _p)

B, C, H, W = 8, 512, 64, 64
N = H * W  # 4096
P = 128
CT = C // P  # 4 channel tiles
KT = N // P  # 32 spatial tiles
FD = 512  # matmul free-dim / PSUM bank width (fp32)
GW = [8, 8, 8, 4, 4]  # k-tiles per load group (sum 32)

_CACHE = {}


def _build_bass():
    import concourse.mybir as mybir
    import concourse.tile as tile
    from concourse import bacc
    from concourse.masks import make_identity

    fp32 = mybir.dt.float32
    bf16 = mybir.dt.bfloat16
    AX = mybir.AxisListType.X
    ALU = mybir.AluOpType
    ACT_EXP = mybir.ActivationFunctionType.Exp
    ACT_COPY = mybir.ActivationFunctionType.Copy

    nc = bacc.Bacc(None, target_bir_lowering=False, debug=False)
    x_d = nc.dram_tensor("x", [C, N], fp32, kind="ExternalInput")
    g_d = nc.dram_tensor("gamma", [1], fp32, kind="ExternalInput")
    o_d = nc.dram_tensor("out", [C, N], fp32, kind="ExternalOutput")

    groups = []
    k0 = 0
    for kw in GW:
        groups.append([(c, k0, kw) for c in range(CT)])
        k0 += kw

    with tile.TileContext(nc) as tc:
        with (
            tc.tile_pool(name="persist", bufs=1) as persist,
            tc.tile_pool(name="etm", bufs=4) as etm,
            tc.tile_pool(name="mnp", bufs=4) as mnp,
            tc.tile_pool(name="sp", bufs=4) as sp,
            tc.tile_pool(name="rgp", bufs=4) as rgp,
            tc.tile_pool(name="outp", bufs=4) as outp,
            tc.tile_pool(name="epsum", bufs=4, space="PSUM") as epsum,
            tc.tile_pool(name="opsum", bufs=3, space="PSUM") as opsum,
            tc.tile_pool(name="atps", bufs=1, space="PSUM") as atps,
        ):
            gam = persist.tile([P, 1], fp32)
            ident = persist.tile([P, P], bf16)
            make_identity(nc, ident)
            ident32 = persist.tile([P, P], fp32)
            make_identity(nc, ident32)
            jt = persist.tile([P, 2], bf16)
            nc.vector.memset(jt, 0.0)
            q = persist.tile([P, CT, N], fp32)
            q_bf = persist.tile([P, CT, N], bf16)
            # k-major qT: qT[p, k, c, v] = q[c*128+v, k*128+p]; energy rhs for
            # chunk k is the contiguous [128, 512] slab qT[:, k, :, :]
            qT = persist.tile([P, KT, CT, P], bf16)
            att = persist.tile([P, CT, C], bf16)
            attT = persist.tile([P, CT, CT, P], bf16)

            nc.gpsimd.dma_start(out=gam, in_=g_d[:].to_broadcast((P, 1)))
            idx = 0
            for grp in groups:
                for c, gk0, kw in grp:
                    sl = slice(gk0 * P, (gk0 + kw) * P)
                    ring = nc.sync if idx % 2 == 0 else nc.scalar
                    ring.dma_start(
                        out=q[:, c, sl], in_=x_d[c * P : (c + 1) * P, sl]
                    )
                    idx += 1

            # HAM warmup: tiny matmuls keep the PE busy-window active while
            # chunk 0 is in flight (warm 2.4GHz clock engages after ~3.4us
            # of sustained activity).
            warm = opsum.tile([P, 2], fp32, name="warm", tag="ops")
            for _ in range(24):
                nc.tensor.matmul(warm[0:1, 0:1], lhsT=jt[:, 0:1], rhs=jt[:, 0:1])

            def cast(ch):
                # on DVE: the ACT engine is also the HWDGE ring sequencer and
                # blocks on dma_start ring space, so load-phase casts must not
                # share its FIFO.
                c, gk0, kw = ch
                sl = slice(gk0 * P, (gk0 + kw) * P)
                nc.vector.tensor_copy(out=q_bf[:, c, sl], in_=q[:, c, sl])

            def pe_transpose(ch):
                c, gk0, kw = ch
                tp = opsum.tile([P, kw * P], bf16, name="tp", tag="ops")
                for kk in range(kw):
                    a = (gk0 + kk) * P
                    nc.tensor.transpose(
                        tp[:, kk * P : (kk + 1) * P], q_bf[:, c, a : a + P], ident
                    )
                return tp

            def gather(ch, tp):
                c, gk0, kw = ch
                nc.vector.tensor_copy(
                    out=qT[:, gk0 : gk0 + kw, c, :],
                    in_=tp.rearrange("p (k v) -> p k v", v=P),
                )

            Es = [
                epsum.tile([P, C], fp32, name=f"E{i}", tag=f"E{i}", bufs=1)
                for i in range(CT)
            ]

            def energy(lo, hi, i_only=None):
                for k in range(lo, hi):
                    for i in range(CT) if i_only is None else [i_only]:
                        nc.tensor.matmul(
                            Es[i][:, i * P :],
                            lhsT=qT[:, k, i, :],
                            rhs=qT[:, k, i:, :],
                            start=(k == 0),
                            stop=(k == KT - 1),
                        )

            # ---- load phase (group-wise: a per-chunk interleave would chain
            # casts behind gathers in the DVE FIFO through the PE) ----
            for gi, grp in enumerate(groups):
                for ch in grp:
                    cast(ch)
                tps = [pe_transpose(ch) for ch in grp]
                for ch, tp in zip(grp, tps):
                    gather(ch, tp)
                if gi < len(groups) - 1:
                    _, gk0, kw = grp[0]
                    energy(gk0, gk0 + kw)

            # ---- tail ----
            tk0 = KT - GW[-1]
            mns = [mnp.tile([P, 1], fp32, name=f"mn{i}", tag="mn") for i in range(CT)]
            sss = [sp.tile([P, 1], fp32, name=f"s{i}", tag="s") for i in range(CT)]
            rgs = [rgp.tile([P, 1], fp32, name=f"rg{i}", tag="rg") for i in range(CT)]

            energy(tk0, KT, i_only=0)
            energy(tk0, KT, i_only=1)
            energy(tk0, KT, i_only=2)
            nc.vector.tensor_reduce(out=mns[0], in_=Es[0], axis=AX, op=ALU.min)
            etmps = {}
            for i in range(1, CT):
                for j in range(i):
                    etmps[(i, j)] = etm.tile([P, P], fp32, name="etmp", tag="etmp")
                    nc.vector.tensor_copy(
                        out=etmps[(i, j)], in_=Es[j][:, i * P : (i + 1) * P]
                    )

            def softmax(i):
                nc.scalar.activation(
                    out=att[:, i, :],
                    in_=Es[i],
                    func=ACT_EXP,
                    bias=mns[i],
                    scale=-1.0,
                    accum_out=sss[i],
                )

            def rgcalc(i):
                nc.vector.reciprocal(out=rgs[i], in_=sss[i])
                nc.vector.tensor_mul(rgs[i], rgs[i], gam)

            softmax(0)
            # attT(0) on the PE: gates the first out matmul. Etail(3) fills
            # the PE while exp(0) runs on ACT.
            atp = atps.tile([P, CT * P], bf16, name="atp", tag="atp")
            for j in range(CT):
                nc.tensor.transpose(
                    atp[:, j * P : (j + 1) * P], att[:, 0, j * P : (j + 1) * P], ident
                )
            energy(tk0, KT, i_only=3)
            rgcalc(0)
            nc.vector.tensor_copy(
                out=attT[:, 0, :, :], in_=atp.rearrange("p (j v) -> p j v", v=P)
            )

            def mirror(i, j):
                nc.tensor.transpose(
                    Es[i][:, j * P : (j + 1) * P], etmps[(i, j)], ident32
                )

            # ---- out = gamma/s * (att @ q) + x ----
            # laggards: (engine, payload) interleaved into block 0's chunks so
            # no engine FIFO ever gates the PSUM rotation.
            def laggards(i, ch):
                if i != 0:
                    return
                if ch == 0:
                    mirror(1, 0)  # PE; after this block's first MM group
                    nc.vector.tensor_reduce(
                        out=mns[1], in_=Es[1], axis=AX, op=ALU.min
                    )
                    softmax(1)  # ACT, waits mirror(1,0) + rowmin(1)
                    nc.sync.dma_start_transpose(
                        out=attT[:, 1, :, :], in_=att[:, 1, :]
                    )
                elif ch == 1:
                    mirror(2, 0)
                    mirror(2, 1)
                    rgcalc(1)
                    nc.vector.tensor_reduce(
                        out=mns[2], in_=Es[2], axis=AX, op=ALU.min
                    )
                elif ch == 2:
                    softmax(2)
                    nc.sync.dma_start_transpose(
                        out=attT[:, 2, :, :], in_=att[:, 2, :]
                    )
                elif ch == 3:
                    mirror(3, 0)
                    mirror(3, 1)
                    mirror(3, 2)
                    rgcalc(2)
                    nc.vector.tensor_reduce(
                        out=mns[3], in_=Es[3], axis=AX, op=ALU.min
                    )
                elif ch == 4:
                    softmax(3)
                    nc.sync.dma_start_transpose(
                        out=attT[:, 3, :, :], in_=att[:, 3, :]
                    )
                elif ch == 5:
                    rgcalc(3)

            NCH = N // FD  # 8 chunks of 512 cols per row-block
            for i in range(CT):
                ot = None
                for ch in range(NCH):
                    sl = slice(ch * FD, (ch + 1) * FD)
                    ops = opsum.tile([P, FD], fp32, name="ops", tag="ops")
                    for j in range(CT):
                        nc.tensor.matmul(
                            ops,
                            lhsT=attT[:, i, j, :],
                            rhs=q_bf[:, j, sl],
                            start=(j == 0),
                            stop=(j == CT - 1),
                        )
                    laggards(i, ch)
                    if ch % 2 == 0:
                        ot = outp.tile([P, 2 * FD], fp32, name="ot", tag="ot")
                    hsl = slice((ch % 2) * FD, (ch % 2 + 1) * FD)
                    # drain ot = psum*(gamma/s) + x, alternating engines so
                    # neither ACT nor DVE gates the PSUM rotation
                    if ch % 2 == 0:
                        nc.scalar.activation(
                            out=ot[:, hsl], in_=ops, func=ACT_COPY, scale=rgs[i]
                        )
                        nc.vector.tensor_add(ot[:, hsl], ot[:, hsl], q[:, i, sl])
                    else:
                        nc.vector.scalar_tensor_tensor(
                            out=ot[:, hsl],
                            in0=ops,
                            scalar=rgs[i],
                            in1=q[:, i, sl],
                            op0=ALU.mult,
                            op1=ALU.add,
                        )
                    if ch % 2 == 1:
                        # sync ring only: the scalar sequencer is the ACT
                        # engine, which must stay free for drains and exps
                        osl = slice((ch - 1) * FD, (ch + 1) * FD)
                        nc.sync.dma_start(out=o_d[i * P : (i + 1) * P, osl], in_=ot)

    nc.compile()
    return nc


def _get_nc():
    if "nc" not in _CACHE:
        _CACHE["nc"] = _build_bass()
    return _CACHE["nc"]


def run(x, gamma, **run_kwargs):
    """Run on 8 cores; returns (results_list, BassKernelResults)."""
    from concourse.bass_utils import run_bass_kernel_spmd

    nc = _get_nc()
    x = np.ascontiguousarray(x, dtype=np.float32)
    gamma = np.ascontiguousarray(gamma, dtype=np.float32)
    in_maps = [
        {"x": np.ascontiguousarray(x[b].reshape(C, N)), "gamma": gamma}
        for b in range(B)
    ]
    res = run_bass_kernel_spmd(nc, in_maps, core_ids=list(range(B)), **run_kwargs)
    out = np.stack([r["out"] for r in res.results]).reshape(B, C, H, W)
    return out, res


def kernel(x, gamma):
    out, _ = run(x, gamma)
    return out.astype(np.float32)
